# revision 25
# baseline (speedup 1.0000x reference)
"""Two-layer heterogeneous GAT (2 edge types) on 8 Trainium2 NeuronCores.

v6 strategy (4-queue SWDGE gathers + host-built one-hot tables):
  - Dst-sharded edge parallelism: edges sorted by 128-node dst block per
    core, padded to per-block-uniform (max over cores) tile counts so one
    SPMD program serves all 8 cores; int16 SWDGE indices need a lo/hi
    stream split (NP > 32768).
  - Node phases are SHARDED: core k computes feat rows [x@W | x@W@Al |
    x@W@Ar] for its own NS nodes, then an AllGather replicates the bf16
    gather table. er stays core-local as [128 node-in-block, NB*H].
  - The per-edge feat[src] gathers are SWDGE dma_gather round-robined
    over num_swdge_queues=4 (the single qPoolDynamic queue drains at only
    ~25 GB/s and was the previous bottleneck; 4 queues scale ~3.5x and the
    Q7 desc-gen also overlaps across queues, ~2.6-3.3 ns/descriptor).
  - The one-hot aggregation matrices M[e,n] (lhsT of the per-tile
    segment-sum U += M^T @ [ee*feat | ee]) and their transposes MT (lhsT
    of the er[dst] fetch erp = MT^T @ er_blk) are STATIC given the graph,
    so they are built on host and DMA-loaded per batch instead of being
    is_equal-built on DVE (which cost ~0.5 ms/layer at 1 elem/cycle).
  - Per-edge er comes from the MT one-hot matmul; ef = el + erp, lrelu,
    exp on ACT, then vals *= ee in place, all per 14/16-tile batch.
  - Edge softmax skips segment-max: logits are bounded so fp32 exp() is
    safe, and the max-subtraction cancels in U/s; pad edges carry dcol
    sentinel -1 which maps to an all-zero one-hot row/column.
  - Block epilogues (softmax normalize, elu, head-mean) are BATCHED over
    groups of 5 dst blocks: U tiles are staged PSUM->SBUF on the idle ACT
    engine and the normalize/elu/head-mean chain runs as a handful of
    wide DVE ops per group instead of ~15 small ops per block.
  - The layer-1 node transform is fused into the layer-0 block loop;
    AllGathers are issued at edge-context entry so table loads overlap.
  - Engine budget per core (measured): gpsimd ~0.7ms (SWDGE desc-gen,
    the current bottleneck), DVE ~0.5ms, PE ~0.3ms, ACT ~0.2ms.
"""

import math
import numpy as np
import ml_dtypes

import concourse.bass as bass
import concourse.bacc as bacc
import concourse.mybir as mybir
import concourse.tile as tile
from concourse.masks import make_identity

F32 = mybir.dt.float32
BF16 = mybir.dt.bfloat16
I16 = mybir.dt.int16
I32 = mybir.dt.int32
I8 = mybir.dt.int8
FP8 = mybir.dt.float8e4
AOT = mybir.AluOpType
ACT = mybir.ActivationFunctionType

P = 128
PAD = -1  # dcol sentinel: matches no node lane (int8 tables)


class CFG:
    def __init__(self, N=50000, E=400000, NC=8, IN=128, H0=4, D0=64, H1=1,
                 D1=64, SLOPE=0.2, KB=16):
        self.N, self.E, self.NC, self.IN = N, E, NC, IN
        self.H0, self.D0, self.H1, self.D1 = H0, D0, H1, D1
        self.SLOPE, self.KB = SLOPE, KB
        NS = math.ceil(N / NC / P) * P
        if NS * NC <= N:
            NS += P
        self.NS = NS                      # nodes per shard (tile aligned)
        self.NP = NS * NC                 # padded total nodes
        self.NB = NS // P                 # dst blocks per core
        self.VD0 = H0 * D0                # 256
        self.VD1 = H1 * D1                # 64
        # bf16 gather rows must be multiples of 128 elems (256B)
        self.C0 = P * math.ceil((self.VD0 + H0) / P)      # 384
        self.C1 = P * math.ceil((self.VD1 + H1) / P)      # 128


def _block_a(al, W):
    """A s.t. (x@W)@A == einsum('nhd,hd->nh', (x@W).reshape(-1,H,D), al)."""
    H, D = al.shape
    A = np.zeros((H * D, H), np.float32)
    for h in range(H):
        A[h * D:(h + 1) * D, h] = al[h]
    return W.astype(np.float32) @ A


def _wrap16(a):
    """Index array -> compact dma_gather idx layout [16, n/16] int16."""
    a = np.asarray(a)
    assert a.min() >= 0 and a.max() <= 32767, (a.min(), a.max())
    a = a.astype(np.int16)
    assert len(a) % 16 == 0
    return a.reshape(-1, 16).T.copy()


def _pick_B(cfg, srcs_dsts):
    """Pick the lo/hi split minimizing total (per-block ragged) tiles."""
    NP, NB, NC = cfg.NP, cfg.NB, cfg.NC
    if NP <= 32768 + P:
        return NP - P
    Bmin = P * math.ceil((NP - 32768) / P)
    grid = [b for b in range(Bmin, 32769, 1024)]
    cost = np.zeros(len(grid), np.int64)
    for (src, dst) in srcs_dsts:
        blk = (dst.astype(np.int64)) >> 7          # == core*NB + local block
        bucket = src.astype(np.int64) >> 10        # 0..48
        cnt = np.bincount(blk * 64 + bucket,
                          minlength=NC * NB * 64).reshape(NC * NB, 64)
        cum = cnt.cumsum(1)
        tot = cum[:, -1]
        for gi, B in enumerate(grid):
            lo = cum[:, B // 1024 - 1].reshape(NC, NB)
            hi = (tot - cum[:, B // 1024 - 1]).reshape(NC, NB)
            tlo = np.maximum(1, -(-lo.max(0) // P))
            thi = np.maximum(1, -(-hi.max(0) // P))
            cost[gi] += tlo.sum() + thi.sum()
    return grid[int(np.argmin(cost))]


def _edge_tables(cfg, src, dst, B):
    """Ragged per-block uniform-tile edge tables for one edge type.

    Returns (TB=[tb_lo, tb_hi] per-block arrays, per_core list over streams
    of dicts with src16 [16, NT*8] and dcr [NCH, 128, 128] bf16)."""
    NC, NS, NB = cfg.NC, cfg.NS, cfg.NB
    src = np.asarray(src, np.int64)
    dst = np.asarray(dst, np.int64)
    blk = dst >> 7
    lo_cnt = np.bincount(blk[src < B], minlength=NC * NB).reshape(NC, NB)
    all_cnt = np.bincount(blk, minlength=NC * NB).reshape(NC, NB)
    hi_cnt = all_cnt - lo_cnt
    TB = [np.maximum(1, -(-lo_cnt.max(0) // P)),
          np.maximum(1, -(-hi_cnt.max(0) // P))]
    OFF = [np.concatenate([[0], t.cumsum()]) for t in TB]
    NT = [int(o[-1]) for o in OFF]

    per_core = []
    for k in range(NC):
        m = (dst >= k * NS) & (dst < (k + 1) * NS)
        s_, d_ = src[m], dst[m] - k * NS
        order = np.argsort(d_ >> 7, kind="stable")
        s_, d_ = s_[order], d_[order]
        bnd = np.searchsorted(d_ >> 7, np.arange(NB + 1))
        streams = []
        for si in range(2):
            n = NT[si] * P
            s16 = np.zeros(n, np.int64)
            dcol = np.full(n, PAD, np.int16)
            for b in range(NB):
                sb, db = s_[bnd[b]:bnd[b + 1]], d_[bnd[b]:bnd[b + 1]]
                lo = sb < B
                if si == 0:
                    sb, db = sb[lo], db[lo]
                else:
                    sb, db = sb[~lo] - B, db[~lo]
                o = OFF[si][b] * P
                s16[o:o + len(sb)] = sb
                dcol[o:o + len(db)] = (db - (b << 7)).astype(np.int16)
            # one-hot M/MT are static: build on host, DMA in at runtime
            lut = np.zeros((P + 1, P), ml_dtypes.bfloat16)
            lut[:P] = np.eye(P, dtype=ml_dtypes.bfloat16)
            dix = dcol.reshape(NT[si], P).astype(np.int64)
            dix = np.where(dix < 0, P, dix)
            m3 = lut[dix]                              # [NT, e, n]
            streams.append({
                # pre-replicated to 128 partitions: per-batch slices load
                # with ONE dma instead of keeping whole streams in SBUF
                "src16": np.tile(_wrap16(s16), (8, 1)),
                "mh": np.ascontiguousarray(
                    m3.transpose(1, 0, 2)).reshape(P, NT[si] * P),
                "mth": np.ascontiguousarray(
                    m3.transpose(2, 0, 1)).reshape(P, NT[si] * P),
            })
        per_core.append(streams)
    return (TB, OFF, NT), per_core


def preprocess(cfg, inputs):
    """All host-side numpy prep. Returns (in_maps, meta)."""
    x = np.asarray(inputs["x"], np.float32)
    W0 = np.asarray(inputs["W0"], np.float32)
    W1 = np.asarray(inputs["W1"], np.float32)
    rhs0 = np.concatenate(
        [W0, _block_a(np.asarray(inputs["al0"]), W0),
         _block_a(np.asarray(inputs["ar0"]), W0)], axis=1)  # [IN, 264]
    rhs1 = np.concatenate(
        [W1, _block_a(np.asarray(inputs["al1"]), W1),
         _block_a(np.asarray(inputs["ar1"]), W1)], axis=1)  # [D0, 66]
    xT = np.zeros((cfg.IN, cfg.NP), np.float32)
    xT[:, :cfg.N] = x.T
    xT = xT.astype(ml_dtypes.bfloat16)
    b0 = np.asarray(inputs["b0"], np.float32)
    b1 = np.asarray(inputs["b1"], np.float32)
    use_b0 = bool(np.any(b0))
    use_b1 = bool(np.any(b1))

    sd = [(np.asarray(inputs["src0"]), np.asarray(inputs["dst0"])),
          (np.asarray(inputs["src1"]), np.asarray(inputs["dst1"]))]
    B = _pick_B(cfg, sd)
    TB, tabs = [], []
    for t in range(2):
        tb, tab = _edge_tables(cfg, sd[t][0], sd[t][1], B)
        TB.append(tb)
        tabs.append(tab)

    in_maps = []
    for k in range(cfg.NC):
        m = {"xTown": np.ascontiguousarray(xT[:, k * cfg.NS:(k + 1) * cfg.NS]),
             "rhs0": rhs0.astype(ml_dtypes.bfloat16),
             "rhs1": rhs1.astype(ml_dtypes.bfloat16)}
        if use_b0:
            m["b0rep"] = np.broadcast_to(b0, (P, cfg.VD0)).copy()
        if use_b1:
            m["b1rep"] = np.broadcast_to(b1, (P, cfg.VD1)).copy()
        for t in range(2):
            for s in range(2):
                st = tabs[t][k][s]
                m[f"s16_{t}{s}"] = st["src16"]
                m[f"mh_{t}{s}"] = st["mh"]
                m[f"mth_{t}{s}"] = st["mth"]
        in_maps.append(m)
    meta = {"TB": TB, "B": B, "use_b0": use_b0, "use_b1": use_b1}
    return in_maps, meta


def build_module(cfg, meta):
    B = meta["B"]
    use_b0, use_b1 = meta["use_b0"], meta["use_b1"]
    VD0, VD1, C0, C1 = cfg.VD0, cfg.VD1, cfg.C0, cfg.C1
    D0, D1, H0, H1 = cfg.D0, cfg.D1, cfg.H0, cfg.H1
    NB, NS, NP = cfg.NB, cfg.NS, cfg.NP
    NT = {(t, s): meta["TB"][t][2][s] for t in range(2) for s in range(2)}

    nc = bacc.Bacc("TRN2", target_bir_lowering=False, num_swdge_queues=4)
    xTo_d = nc.declare_dram_parameter("xTown", [cfg.IN, NS], BF16,
                                      isOutput=False)
    rhs0_d = nc.declare_dram_parameter("rhs0", [cfg.IN, VD0 + 2 * H0], BF16,
                                       isOutput=False)
    rhs1_d = nc.declare_dram_parameter("rhs1", [D0, VD1 + 2 * H1], BF16,
                                       isOutput=False)
    b0_d = b1_d = None
    if use_b0:
        b0_d = nc.declare_dram_parameter("b0rep", [P, VD0], F32,
                                         isOutput=False)
    if use_b1:
        b1_d = nc.declare_dram_parameter("b1rep", [P, VD1], F32,
                                         isOutput=False)
    s16_d, mh_d, mth_d = {}, {}, {}
    for t in range(2):
        for s in range(2):
            n = NT[t, s]
            s16_d[t, s] = nc.declare_dram_parameter(
                f"s16_{t}{s}", [P, n * 8], I16, isOutput=False)
            mh_d[t, s] = nc.declare_dram_parameter(
                f"mh_{t}{s}", [P, n * P], BF16, isOutput=False)
            mth_d[t, s] = nc.declare_dram_parameter(
                f"mth_{t}{s}", [P, n * P], BF16, isOutput=False)
    out_d = nc.declare_dram_parameter("out", [NS, D1], BF16, isOutput=True)

    feat0_own = nc.dram_tensor("feat0_own", [NS, C0], BF16)
    feat0_all = nc.dram_tensor("feat0_all", [cfg.NC, NS, C0], BF16,
                               addr_space="Shared")
    feat1_own = nc.dram_tensor("feat1_own", [NS, C1], BF16)
    feat1_all = nc.dram_tensor("feat1_all", [cfg.NC, NS, C1], BF16,
                               addr_space="Shared")
    er0_d = nc.dram_tensor("er0_d", [P, NB * H0], BF16)
    er1_d = nc.dram_tensor("er1_d", [P, NB * H1], BF16)

    # ---------------- Layer-0 node phase (sharded) -------------
    with tile.TileContext(nc) as tc:
        node_phase(nc, tc, cfg, cfg.IN, VD0, H0, C0,
                   lambda b: xTo_d[:, b * P:(b + 1) * P],
                   rhs0_d, feat0_own, er0_d)

    # --- Layer-0 edge phase; per-block fused layer-1 node transform -------
    ag0 = (feat0_own, feat0_all)
    flat0 = feat0_all[:, :, :].rearrange("c n e -> (c n) e")
    with tile.TileContext(nc) as tc:
        edge_phase(nc, tc, cfg, meta, s16_d, mh_d, mth_d,
                   flat0[0:B, :], flat0[B:NP, :], er0_d,
                   C0, VD0, H0, D0, l0=True, b_d=b0_d,
                   out_d=None, ag=ag0, kb=14,
                   l1_node=(rhs1_d, feat1_own, er1_d, VD1, H1))

    # ---------------- Layer-1 edge phase (AllGather overlaps prep) --------
    ag1 = (feat1_own, feat1_all)
    flat1 = feat1_all[:, :, :].rearrange("c n e -> (c n) e")
    with tile.TileContext(nc) as tc:
        edge_phase(nc, tc, cfg, meta, s16_d, mh_d, mth_d,
                   flat1[0:B, :], flat1[B:NP, :], er1_d,
                   C1, VD1, H1, D1, l0=False, b_d=b1_d,
                   out_d=out_d, ag=ag1, kb=16)
    nc.compile()
    return nc


def node_phase(nc, tc, cfg, kdim, VD, H, C, lhs_fn, rhs_d, feat_own, er_d,
               lhs_dt=BF16):
    """Sharded node phase: rows [P, VD+H] = lhsT_b^T @ [W|Al|Ar]; feat+el
    -> feat_own rows (stride C), er -> er_d [128, NB*H]."""
    W = VD + 2 * H
    with (
        tc.tile_pool(name="np_c", bufs=1) as cp,
        tc.tile_pool(name="np_sb", bufs=4) as sp,
        tc.tile_pool(name="np_ps", bufs=3, space="PSUM") as pp,
    ):
        rhs_sb = cp.tile([kdim, W], BF16)
        nc.sync.dma_start(out=rhs_sb[:], in_=rhs_d[:, :])
        er_acc = cp.tile([P, cfg.NB * H], BF16)
        for b in range(cfg.NB):
            lt = sp.tile([kdim, P], BF16, tag="lhs")
            if lhs_dt == BF16:
                nc.sync.dma_start(out=lt[:], in_=lhs_fn(b))
            else:
                l8 = sp.tile([kdim, P], lhs_dt, tag="lhs8")
                nc.sync.dma_start(out=l8[:], in_=lhs_fn(b))
                nc.vector.tensor_copy(out=lt[:], in_=l8[:])
            ps = pp.tile([P, W], F32)
            nc.tensor.matmul(out=ps[:], lhsT=lt[:], rhs=rhs_sb[:],
                             start=True, stop=True)
            st = sp.tile([P, VD + H], BF16, tag="stage")
            if b % 2 == 0:
                nc.vector.tensor_copy(out=st[:], in_=ps[:, 0:VD + H])
            else:
                nc.scalar.activation(out=st[:], in_=ps[:, 0:VD + H],
                                     func=ACT.Copy)
            nc.sync.dma_start(out=feat_own[b * P:(b + 1) * P, 0:VD + H],
                              in_=st[:])
            nc.vector.tensor_copy(out=er_acc[:, b * H:(b + 1) * H],
                                  in_=ps[:, VD + H:W])
        nc.sync.dma_start(out=er_d[:, :], in_=er_acc[:])


def edge_phase(nc, tc, cfg, meta, s16_d, mh_d, mth_d, tab_lo, tab_hi, er_d,
               C, VD, H, D, l0, b_d, out_d, ag=None, l1_node=None, kb=None):
    """Edge softmax + aggregation for both edge types, block by block."""
    NB, SLOPE = cfg.NB, cfg.SLOPE
    KB = kb or cfg.KB
    if ag is not None:
        # feat AllGather issued here so the table loads below overlap it;
        # the dma_gathers depend on its output and are ordered after it
        nc.gpsimd.collective_compute(
            "AllGather", AOT.bypass,
            replica_groups=[list(range(cfg.NC))],
            ins=[ag[0][:, :]],
            outs=[ag[1][:, :, :]],
        )
    TB = {(t, s): meta["TB"][t][0][s] for t in range(2) for s in range(2)}
    OFF = {(t, s): meta["TB"][t][1][s] for t in range(2) for s in range(2)}
    NT = {(t, s): meta["TB"][t][2][s] for t in range(2) for s in range(2)}
    blk_of = {k: np.repeat(np.arange(NB), TB[k]) for k in TB}

    with (
        tc.tile_pool(name="e_tab", bufs=1) as tp,
        tc.tile_pool(name="e_s16", bufs=2) as sp16,
        tc.tile_pool(name="e_g", bufs=2) as gp,
        tc.tile_pool(name="e_mt", bufs=2) as mtp,
        tc.tile_pool(name="e_ee", bufs=2) as eep,
        tc.tile_pool(name="e_m", bufs=2) as mp,
        tc.tile_pool(name="e_ep", bufs=2) as epi,
        tc.tile_pool(name="e_ups", bufs=4, space="PSUM") as up,
        tc.tile_pool(name="e_erp", bufs=2, space="PSUM") as erpp,
        tc.tile_pool(name="e_tp", bufs=2, space="PSUM") as tpp,
    ):
        # --- constants ---
        ident = tp.tile([P, P], F32, tag="ident")
        make_identity(nc, ident[:])
        er_sb = tp.tile([P, NB * H], BF16, tag="er")
        nc.sync.dma_start(out=er_sb[:], in_=er_d[:, :])
        if b_d is not None:
            b_sb = tp.tile([P, VD], F32, tag="bias")
            nc.sync.dma_start(out=b_sb[:], in_=b_d[:, :])
        if l0:
            hT_sb = tp.tile([D, cfg.NS], BF16, tag="hT")
        if l1_node is not None:
            rhs1_d, feat1_own, er1_d, VD1, H1 = l1_node
            W1 = VD1 + 2 * H1
            rhs1_sb = tp.tile([D, W1], BF16, tag="rhs1")
            nc.sync.dma_start(out=rhs1_sb[:], in_=rhs1_d[:, :])
            er1_acc = tp.tile([P, NB * H1], BF16, tag="er1a")

        state = {}
        qctr = [0]

        def ensure_batch(t, s, slot):
            bi = slot // KB
            st = state.get((t, s))
            if st is not None and st["bi"] == bi:
                return st
            K = min(KB, NT[t, s] - bi * KB)
            s16b = sp16.tile([P, KB * 8], I16, tag=f"s16{t}{s}")
            nc.sync.dma_start(
                out=s16b[:, 0:K * 8],
                in_=s16_d[t, s][:, bi * KB * 8:(bi * KB + K) * 8])
            g = gp.tile([P, KB * C], BF16, tag=f"g{t}{s}")
            g3 = g[:].rearrange("p (k c) -> p k c", k=KB)
            nc.gpsimd.dma_gather(
                out_ap=g3[:, 0:K, :],
                in_ap=(tab_lo if s == 0 else tab_hi),
                idxs_ap=s16b[:, 0:K * 8],
                num_idxs=K * P, num_idxs_reg=K * P, elem_size=C,
                single_packet=False, queue_num=qctr[0] % 4)
            qctr[0] += 1
            # static one-hot M / MT: prebuilt on host, plain DMA loads
            mba = mp.tile([P, KB * P], BF16, tag=f"mba{t}{s}")
            nc.sync.dma_start(
                out=mba[:, 0:K * P],
                in_=mh_d[t, s][:, bi * KB * P:(bi * KB + K) * P])
            mt = mtp.tile([P, KB * P], BF16, tag=f"mt{t}{s}")
            nc.sync.dma_start(
                out=mt[:, 0:K * P],
                in_=mth_d[t, s][:, bi * KB * P:(bi * KB + K) * P])
            erp = erpp.tile([P, KB * H], F32, tag="erp")
            for k in range(K):
                blk = int(blk_of[t, s][bi * KB + k])
                nc.tensor.matmul(out=erp[:, k * H:(k + 1) * H],
                                 lhsT=mt[:, k * P:(k + 1) * P],
                                 rhs=er_sb[:, blk * H:(blk + 1) * H],
                                 start=True, stop=True)
            ef = eep.tile([P, KB * H], F32, tag=f"ef{t}{s}")
            nc.vector.tensor_tensor(out=ef[:, 0:K * H],
                                    in0=g3[:, 0:K, VD:VD + H],
                                    in1=erp[:, 0:K * H], op=AOT.add)
            nc.vector.scalar_tensor_tensor(
                out=ef[:, 0:K * H], in0=ef[:, 0:K * H], scalar=SLOPE,
                in1=ef[:, 0:K * H], op0=AOT.mult, op1=AOT.max)
            # ee overwrites the el columns of g, so [vals | ee] is one
            # contiguous matmul rhs per tile
            nc.scalar.activation(out=g3[:, 0:K, VD:VD + H],
                                 in_=ef[:, 0:K * H], func=ACT.Exp)
            # vals in place: g.feat *= ee (broadcast over D)
            nc.vector.tensor_tensor(
                out=g3[:, 0:K, 0:VD].rearrange("p k (h d) -> p k h d", h=H),
                in0=g3[:, 0:K, 0:VD].rearrange("p k (h d) -> p k h d", h=H),
                in1=g3[:, 0:K, VD:VD + H, None].to_broadcast([P, K, H, D]),
                op=AOT.mult)
            st = {"bi": bi, "g3": g3, "mba": mba}
            state[(t, s)] = st
            return st

        GE = 5               # blocks per grouped epilogue
        CW = VD + H
        usb = None
        for b in range(NB):
            gi = b % GE
            if gi == 0:
                usb = epi.tile([P, GE * 2 * CW], F32, tag="usb")
            for t in range(2):
                u = up.tile([P, CW], F32, tag="u")
                first = True
                for s in range(2):
                    for j in range(int(TB[t, s][b])):
                        slot = int(OFF[t, s][b]) + j
                        stt = ensure_batch(t, s, slot)
                        kk = slot - stt["bi"] * KB
                        last = (s == 1 and j == TB[t, 1][b] - 1)
                        nc.tensor.matmul(
                            out=u[:, 0:CW],
                            lhsT=stt["mba"][:, kk * P:(kk + 1) * P],
                            rhs=stt["g3"][:, kk, 0:CW],
                            start=first, stop=last)
                        first = False
                # stage U out of PSUM on the (idle) ACT engine
                q = gi * 2 + t
                nc.scalar.activation(out=usb[:, q * CW:(q + 1) * CW],
                                     in_=u[:, 0:CW], func=ACT.Copy)

            if gi != GE - 1 and b != NB - 1:
                continue
            # ---- grouped epilogue over ng blocks ----
            ng = gi + 1
            b0_ = b - gi
            u3 = usb[:, 0:ng * 2 * CW].rearrange("p (q c) -> p q c",
                                                 q=ng * 2)
            sm = epi.tile([P, GE * 2 * H], F32, tag="sm")
            nc.vector.tensor_scalar(
                out=sm[:, 0:ng * 2 * H].rearrange("p (q h) -> p q h",
                                                  q=ng * 2),
                in0=u3[:, :, VD:VD + H], scalar1=1e-9, scalar2=None,
                op0=AOT.max)
            rc = epi.tile([P, GE * 2 * H], F32, tag="rc")
            nc.vector.reciprocal(out=rc[:, 0:ng * 2 * H],
                                 in_=sm[:, 0:ng * 2 * H])
            nc.vector.tensor_tensor(
                out=u3[:, :, 0:VD].rearrange("p q (h d) -> p q h d", h=H),
                in0=u3[:, :, 0:VD].rearrange("p q (h d) -> p q h d", h=H),
                in1=rc[:, 0:ng * 2 * H].rearrange(
                    "p (q h) -> p q h", q=ng * 2)[:, :, :, None
                                                  ].to_broadcast(
                    [P, ng * 2, H, D]),
                op=AOT.mult)
            if b_d is not None:
                nc.vector.tensor_tensor(
                    out=u3[:, :, 0:VD], in0=u3[:, :, 0:VD],
                    in1=b_sb[:, None, :].to_broadcast([P, ng * 2, VD]),
                    op=AOT.add)
            if l0:
                sv = u3[:, :, 0:VD]
                scr = epi.tile([P, GE * 2 * VD], F32, tag="scr")
                sc2 = scr[:, 0:ng * 2 * VD].rearrange("p (q v) -> p q v",
                                                      q=ng * 2)
                nc.vector.tensor_scalar(out=sc2, in0=sv, scalar1=0.0,
                                        scalar2=None, op0=AOT.min)
                nc.scalar.activation(out=scr[:, 0:ng * 2 * VD],
                                     in_=scr[:, 0:ng * 2 * VD], func=ACT.Exp)
                nc.vector.scalar_tensor_tensor(
                    out=sc2, in0=sv, scalar=0.0, in1=sc2,
                    op0=AOT.max, op1=AOT.add)
                # sum the two types in place, then mean over heads
                s4 = scr[:, 0:ng * 2 * VD].rearrange(
                    "p (q u v) -> p q u v", q=ng, u=2)
                nc.vector.tensor_tensor(out=s4[:, :, 0, :],
                                        in0=s4[:, :, 0, :],
                                        in1=s4[:, :, 1, :], op=AOT.add)
                h4 = s4[:, :, 0, :].rearrange("p q (h d) -> p q h d", h=H)
                pair = epi.tile([P, GE * 2 * D], F32, tag="pair")
                pr3 = pair[:, 0:ng * 2 * D].rearrange(
                    "p (q u d) -> p q u d", q=ng, u=2)
                nc.vector.tensor_tensor(out=pr3[:, :, 0, :],
                                        in0=h4[:, :, 0, :],
                                        in1=h4[:, :, 1, :], op=AOT.add)
                nc.vector.tensor_tensor(out=pr3[:, :, 1, :],
                                        in0=h4[:, :, 2, :],
                                        in1=h4[:, :, 3, :], op=AOT.add)
                hb_g = epi.tile([P, GE * D], F32, tag="hbg")
                hb3 = hb_g[:, 0:ng * D].rearrange("p (q d) -> p q d", q=ng)
                nc.vector.tensor_tensor(out=hb3, in0=pr3[:, :, 0, :],
                                        in1=pr3[:, :, 1, :], op=AOT.add)
                # mean over heads of (elu+1) summed over 2 types -> -2
                nc.vector.tensor_scalar(out=hb_g[:, 0:ng * D],
                                        in0=hb_g[:, 0:ng * D],
                                        scalar1=1.0 / H, scalar2=-2.0,
                                        op0=AOT.mult, op1=AOT.add)
                for g in range(ng):
                    blk = b0_ + g
                    tps = tpp.tile([D, P], F32, tag="tp")
                    nc.tensor.transpose(out=tps[:],
                                        in_=hb_g[:, g * D:(g + 1) * D],
                                        identity=ident[:])
                    nc.scalar.activation(
                        out=hT_sb[:, blk * P:(blk + 1) * P],
                        in_=tps[:], func=ACT.Copy)
                    if l1_node is not None:
                        ps1 = up.tile([P, CW], F32, tag="u")
                        nc.tensor.matmul(
                            out=ps1[:, 0:W1],
                            lhsT=hT_sb[:, blk * P:(blk + 1) * P],
                            rhs=rhs1_sb[:], start=True, stop=True)
                        st1 = epi.tile([P, VD1 + H1], BF16, tag="st1")
                        nc.scalar.activation(out=st1[:],
                                             in_=ps1[:, 0:VD1 + H1],
                                             func=ACT.Copy)
                        nc.sync.dma_start(
                            out=feat1_own[blk * P:(blk + 1) * P,
                                          0:VD1 + H1],
                            in_=st1[:])
                        nc.scalar.activation(
                            out=er1_acc[:, blk * H1:(blk + 1) * H1],
                            in_=ps1[:, VD1 + H1:W1], func=ACT.Copy)
            else:
                u5 = usb[:, 0:ng * 2 * CW].rearrange(
                    "p (q u c) -> p q u c", q=ng, u=2)
                os_g = epi.tile([P, GE * D], BF16, tag="osg")
                nc.vector.tensor_tensor(
                    out=os_g[:, 0:ng * D].rearrange("p (q d) -> p q d",
                                                    q=ng),
                    in0=u5[:, :, 0, 0:VD], in1=u5[:, :, 1, 0:VD],
                    op=AOT.add)
                nc.sync.dma_start(
                    out=out_d[b0_ * P:(b + 1) * P, :].rearrange(
                        "(q p) d -> p q d", p=P),
                    in_=os_g[:, 0:ng * D].rearrange("p (q d) -> p q d",
                                                    q=ng))
        if l1_node is not None:
            nc.sync.dma_start(out=er1_d[:, :], in_=er1_acc[:])


def _run_pjrt_lean(nc, in_maps, n_cores):
    """run_bass_via_pjrt fork: output donation buffers are created on-device
    (jnp.zeros under jit) instead of host-staged np.zeros."""
    import jax
    import jax.numpy as jnp
    from jax.sharding import Mesh, PartitionSpec, NamedSharding
    from jax.experimental.shard_map import shard_map
    from concourse import bass2jax
    from concourse.bass2jax import _bass_exec_p, partition_id_tensor

    bass2jax.install_neuronx_cc_hook()
    partition_name = (nc.partition_id_tensor.name
                      if nc.partition_id_tensor else None)
    in_names, out_names, out_avals = [], [], []
    for alloc in nc.m.functions[0].allocations:
        if not isinstance(alloc, mybir.MemoryLocationSet):
            continue
        name = alloc.memorylocations[0].name
        if alloc.kind == "ExternalInput":
            if name != partition_name:
                in_names.append(name)
        elif alloc.kind == "ExternalOutput":
            out_names.append(name)
            out_avals.append(jax.core.ShapedArray(
                tuple(alloc.tensor_shape), mybir.dt.np(alloc.dtype)))
    n_params, n_outs = len(in_names), len(out_avals)
    all_names = in_names + out_names + ([partition_name]
                                        if partition_name else [])

    def _body(*args):
        ops = list(args)
        if partition_name is not None:
            ops.append(partition_id_tensor())
        return tuple(_bass_exec_p.bind(
            *ops, out_avals=tuple(out_avals), in_names=tuple(all_names),
            out_names=tuple(out_names), lowering_input_output_aliases=(),
            sim_require_finite=True, sim_require_nnan=True, nc=nc))

    devices = jax.devices()[:n_cores]
    mesh = Mesh(np.asarray(devices), ("core",))
    sh = NamedSharding(mesh, PartitionSpec("core"))
    sharded = jax.jit(
        shard_map(_body, mesh=mesh,
                  in_specs=(PartitionSpec("core"),) * (n_params + n_outs),
                  out_specs=(PartitionSpec("core"),) * n_outs,
                  check_rep=False),
        donate_argnums=tuple(range(n_params, n_params + n_outs)),
        keep_unused=True)
    concat_in = [np.concatenate([np.asarray(m[nm]) for m in in_maps], axis=0)
                 for nm in in_names]
    zshapes = [(n_cores * a.shape[0], *a.shape[1:]) for a in out_avals]
    zdt = [a.dtype for a in out_avals]
    zfn = jax.jit(
        lambda: tuple(jnp.zeros(s, d) for s, d in zip(zshapes, zdt)),
        out_shardings=(sh,) * n_outs)
    zeros_dev = zfn()
    out_arrs = sharded(*concat_in, *zeros_dev)
    return [
        {name: np.asarray(out_arrs[i]).reshape(
            n_cores, *out_avals[i].shape)[c]
         for i, name in enumerate(out_names)}
        for c in range(n_cores)
    ]


def run(cfg, inputs, core_ids=None, sim=False, trace=False):
    from concourse.bass_utils import run_bass_kernel_spmd
    in_maps, meta = preprocess(cfg, inputs)
    nc = build_module(cfg, meta)
    if sim:
        from concourse.bass_interp import MultiCoreSim
        ms = MultiCoreSim(nc, cfg.NC, require_finite=False, require_nnan=False)
        for k in range(cfg.NC):
            for name, arr in in_maps[k].items():
                ms.cores[k].tensor(name)[:] = arr
        ms.simulate()
        print(f"sim global_time: {ms.global_time} ns")
        results = [{"out": ms.cores[k].tensor("out").copy()}
                   for k in range(cfg.NC)]
        res = None
    elif trace:
        if core_ids is None:
            core_ids = list(range(cfg.NC))
        res = run_bass_kernel_spmd(nc, in_maps, core_ids, trace=trace)
        results = res.results
    else:
        res = None
        try:
            results = _run_pjrt_lean(nc, in_maps, cfg.NC)
        except Exception:
            if core_ids is None:
                core_ids = list(range(cfg.NC))
            res = run_bass_kernel_spmd(nc, in_maps, core_ids)
            results = res.results
    out = np.concatenate([np.asarray(results[k]["out"], np.float32)
                          for k in range(cfg.NC)], axis=0)[:cfg.N]
    return out, res


def kernel(**inputs) -> np.ndarray:
    cfg = CFG()
    out, _ = run(cfg, inputs)
    return out.astype(np.float32)

